# revision 18
# baseline (speedup 1.0000x reference)
# MultiLoraConv2d kernel for 8 trn2 NeuronCores (Bass/Tile, data-parallel over batch).
#
# Math (per sample b):
#   delta_flat[b] = sum_t 2*alphas[b,t] * (lora_B[t] @ lora_A[t])        [768, 768]
#   agg[b] = W + delta_flat[b].reshape(COUT, CIN, 3, 3)                  (flat reinterpret)
#   out[b] = conv2d(x[b], agg[b], pad=1)
#
# Device strategy (per core, S = B/8 samples):
#   - All PE operands bf16: fp32 LDWEIGHTS (224ns) only partially hides under
#     the 213ns N=512 matmul stream (286ns/MM measured); bf16 FWL loads hide
#     fully (216ns/MM cadence measured).
#   - Host pre-lays-out tensors partition-first; LoRA factors regrouped by
#     (d = 3*i + j, s = (c*9+d)//768) so per-sample aggregated conv weights
#     come out of the PE directly in c-major (stationary) layout:
#       dp_d[c, smp, o] = sum_s sum_r a3[d,s,r,c] * (2*alpha_{t(r)} * b3[s,r,o])
#   - +W fold: most dp banks evicted with a DVE tensor-add (dp + wt -> wsf
#     bf16); every ~9th dp instead gets an identity-stationary matmul
#     accumulating W in PSUM and a Scalar-engine copy eviction, balancing
#     PE/DVE/ACT so phase 1 stays PE-bound.
#   - Conv = 18 shifted matmuls (9 taps x 2 cin tiles) accumulated per PSUM
#     bank, x zero-padded (66x66) bf16 in SBUF; evictions alternate
#     Vector/Scalar engines.
import numpy as np

B, T, R, ALPHA = 32, 4, 8, 16
CIN, COUT, K = 256, 256, 3
H = W_SP = 64
SCALING = ALPHA / R
NCORES = 8
S = B // NCORES      # samples per core
NR = T * R * K       # 96 lora rows (padded to 128 partitions)
P = 128
HP = H + 2           # 66 padded
HH = 34              # padded-row half-tile height (rows 0:34 and 32:66)
HSMP = S // 2        # samples per 512-wide delta-matmul half
EYES = {8, 17, 26, 35}  # dp indices evicted via eye-matmul + Scalar copy

_CACHE = {}


def _build_nc():
    import concourse.bacc as bacc
    import concourse.mybir as mybir
    import concourse.tile as tile

    f32 = mybir.dt.float32
    bf16 = mybir.dt.bfloat16

    nc = bacc.Bacc("TRN2", target_bir_lowering=False, debug=False, num_devices=NCORES)

    xp = nc.declare_dram_parameter("xp", [S, 2, P, HP, HP], bf16, isOutput=False)
    wt = nc.declare_dram_parameter("wt", [P, 9, 2, COUT], bf16, isOutput=False)
    a3 = nc.declare_dram_parameter("a3", [P, 9, 3, CIN], bf16, isOutput=False)
    b3 = nc.declare_dram_parameter("b3", [P, 3, COUT], f32, isOutput=False)
    alph = nc.declare_dram_parameter("alph", [P, S], f32, isOutput=False)
    eye = nc.declare_dram_parameter("eye", [P, P], bf16, isOutput=False)
    outd = nc.declare_dram_parameter("out", [S, 2, P, H, W_SP], f32, isOutput=True)

    with tile.TileContext(nc) as tc:
        with tc.tile_pool(name="ws_pool", bufs=2) as ws_pool, \
             tc.tile_pool(name="xt_pool", bufs=8) as xt_pool:
            # per-cin-tile aggregated conv weights: [c, d, half, i, o] bf16
            wsf = [ws_pool.tile([P, 9, 2, 2, COUT], bf16, name="wsf")
                   for _ in range(2)]

            # ---- phase 1: aggregated weights via LoRA matmuls ----
            with tc.tile_pool(name="cst", bufs=1) as cst, \
                 tc.tile_pool(name="wps", bufs=1, space="PSUM") as wps, \
                 tc.tile_pool(name="dps", bufs=7, space="PSUM") as dps:
                a3_sb = cst.tile([P, 9, 3, CIN], bf16)
                b3_sb = cst.tile([P, 3, COUT], f32)
                alph_sb = cst.tile([P, S], f32)
                wt_sb = cst.tile([P, 9, 2, COUT], bf16)
                eye_sb = cst.tile([P, P], bf16)
                warm = cst.tile([P, P], bf16)
                b3s = [cst.tile([P, 3, HSMP, COUT], bf16, name=f"b3s{h}")
                       for h in range(2)]

                # DMA transfers complete strictly in issue order at ~290GB/s
                # (each dma_start fans over all 16 engines). Issue in the
                # order the pipeline consumes: small consts, then per-tap
                # a3/wt pairs (a3 gates the delta matmuls, wt the +W
                # accumulation), then sample-0 x.
                # b3/alph ride the Scalar queue (free at ~5.6us) so the
                # b3s muls can start ~1.5us earlier than via the sync queue.
                nc.scalar.dma_start(b3_sb[:, :, :], b3[:, :, :])
                nc.scalar.dma_start(alph_sb[:, :], alph[:, :])
                nc.sync.dma_start(eye_sb[:, :], eye[:, :])
                nc.sync.dma_start(a3_sb[:, 0:1], a3[:, 0:1])
                nc.sync.dma_start(wt_sb[:, 0:1], wt[:, 0:1])
                nc.sync.dma_start(a3_sb[:, 1:3], a3[:, 1:3])
                nc.sync.dma_start(wt_sb[:, 1:3], wt[:, 1:3])
                nc.sync.dma_start(a3_sb[:, 3:6], a3[:, 3:6])
                nc.sync.dma_start(wt_sb[:, 3:6], wt[:, 3:6])
                nc.sync.dma_start(a3_sb[:, 6:9], a3[:, 6:9])
                nc.sync.dma_start(wt_sb[:, 6:9], wt[:, 6:9])
                # sample-0 x as (ct, h-half) quarters, lower halves first so
                # the first conv bank-groups can start before the rest lands
                xt0 = [[None, None], [None, None]]
                for h in range(2):
                    for ct in range(2):
                        t = xt_pool.tile([P, HH, HP], bf16, name="xt")
                        nc.sync.dma_start(t[:, :, :], xp[0, ct, :, 32 * h:32 * h + HH, :])
                        xt0[ct][h] = t

                # HAM warmup: PE busy from ~6.5us so the clock gate flips
                # to 8/8 and the first-DMA spin-up (~10.5us to b3s ready)
                # is covered. memset is the first GpSimd instruction.
                nc.gpsimd.memset(warm[:, :], 0.0)
                wp = wps.tile([P, P], f32, name="wp")
                for _ in range(40):
                    nc.tensor.matmul(wp[:, :], warm[:, :], warm[:, :],
                                     start=True, stop=True)

                # b3s[half][r, s, smp, o] = b3[r, s, o] * 2*alpha[smp, t(r)]
                # (2x folded into alph host-side). DVE/GpSimd tensor_scalar
                # with an AP scalar hits a ~3.9us/op slow path after the
                # first op, so: half0 as DVE tensor_tensor with a broadcast
                # alpha (s-major, so the first delta matmuls start asap),
                # half1 as Scalar ACTIVATE Copy-with-scale — in parallel.
                # interleave engines by sample so each s-pair completes as
                # the delta-matmul stream reaches it
                for half in range(2):
                    for s in range(3):
                        nc.vector.tensor_mul(
                            b3s[half][:, s, 0, :], b3_sb[:, s, :],
                            alph_sb[:, 2 * half:2 * half + 1]
                            .to_broadcast([P, COUT]))
                        nc.scalar.mul(
                            b3s[half][:, s, 1, :], b3_sb[:, s, :],
                            alph_sb[:, 2 * half + 1:2 * half + 2])

                ev = 0
                for half in range(2):
                    for d in range(9):
                        for ct in range(2):
                            use_eye = ev in EYES
                            dp = dps.tile([P, HSMP, COUT], f32, name="dp")
                            for s in range(3):
                                nc.tensor.matmul(
                                    dp[:, :, :],
                                    a3_sb[:, d, s, ct * P:(ct + 1) * P],
                                    b3s[half][:, s, :, :],
                                    start=(s == 0),
                                    stop=(s == 2 and not use_eye))
                            wtb = wt_sb[:, d, ct, None, :].to_broadcast(
                                [P, HSMP, COUT])
                            dst = wsf[ct][:, d, half, :, :]
                            if use_eye:
                                # += W via identity stationary, wt broadcast
                                # over the sample axis as the moving operand
                                nc.tensor.matmul(
                                    dp[:, :, :], eye_sb[:, :], wtb,
                                    start=False, stop=True)
                                nc.scalar.copy(dst, dp[:, :, :])
                            else:
                                nc.vector.tensor_add(dst, dp[:, :, :], wtb)
                            ev += 1

            # ---- phase 2: per-sample conv, 18 shifted matmuls per psum bank ----
            with tc.tile_pool(name="ob_pool", bufs=4) as ob_pool, \
                 tc.tile_pool(name="cps", bufs=8, space="PSUM") as cps:
                for smp in range(S):
                    half, i = divmod(smp, HSMP)
                    if smp == 0:
                        xts = xt0
                    else:
                        xts = [[None, None], [None, None]]
                        for h in range(2):
                            for ct in range(2):
                                t = xt_pool.tile([P, HH, HP], bf16, name="xt")
                                nc.sync.dma_start(
                                    t[:, :, :], xp[smp, ct, :, 32 * h:32 * h + HH, :])
                                xts[ct][h] = t
                    # weight-stationary over 4-bank groups: the same wsf
                    # slice feeds 4 consecutive matmuls (one per bank), so
                    # the LDWEIGHTS can be elided/pipelined by codegen.
                    for ot in range(2):
                        for g in range(2):
                            pbs = [cps.tile([P, 8, W_SP], f32, name="pb")
                                   for _ in range(4)]
                            for ct in range(2):
                                for d in range(9):
                                    di, dj = divmod(d, 3)
                                    for k in range(4):
                                        hb = g * 4 + k
                                        loc = (hb % 4) * 8 + di
                                        nc.tensor.matmul(
                                            pbs[k][:, :, :],
                                            wsf[ct][:, d, half, i,
                                                    ot * P:(ot + 1) * P],
                                            xts[ct][hb // 4][:, loc:loc + 8,
                                                             dj:dj + W_SP],
                                            start=(ct == 0 and d == 0),
                                            stop=(ct == 1 and d == 8))
                            for k in range(4):
                                hb = g * 4 + k
                                ob = ob_pool.tile([P, 8, W_SP], f32, name="ob")
                                if k % 2 == 0:
                                    nc.vector.tensor_copy(ob[:, :, :],
                                                          pbs[k][:, :, :])
                                else:
                                    nc.scalar.copy(ob[:, :, :], pbs[k][:, :, :])
                                nc.sync.dma_start(
                                    outd[smp, ot, :, hb * 8:(hb + 1) * 8, :],
                                    ob[:, :, :])
    nc.finalize()
    return nc


def _host_prep(x, alphas, W, lora_A, lora_B):
    """Host-side layout-only transforms (pad/transpose/gather/replicate)."""
    import ml_dtypes
    bf = ml_dtypes.bfloat16

    xf = np.ascontiguousarray(np.asarray(x, dtype=np.float32))
    af = np.asarray(alphas, dtype=np.float32)
    Wf = np.asarray(W, dtype=np.float32)
    Af = np.asarray(lora_A, dtype=np.float32).reshape(NR, CIN * K)   # Acat
    Bf = np.asarray(lora_B, dtype=np.float32)

    # padded x, per core: (S, 2, 128, 66, 66) bf16
    xpad = np.zeros((B, CIN, HP, HP), bf)
    xpad[:, :, 1:-1, 1:-1] = xf.astype(bf)
    xpad = xpad.reshape(NCORES, S, 2, P, HP, HP)

    # base weights c-major, d-major free layout: wt[p, d, ct, o]
    wth = np.ascontiguousarray(
        Wf.reshape(COUT, CIN, 9).transpose(1, 2, 0)        # [c, d, o]
        .reshape(2, P, 9, COUT)                            # [ct, p, d, o]
        .transpose(1, 2, 0, 3)).astype(bf)                 # [p, d, ct, o]

    # a3[r, d, s, c] = Acat[r, c*9+d-768*s] masked; rows padded 96 -> 128
    a3h = np.zeros((P, 9, 3, CIN), np.float32)
    cc = np.arange(CIN)
    for d in range(9):
        q = cc * 9 + d
        s_of_c = q // (CIN * K)
        q_of_c = q % (CIN * K)
        for s in range(3):
            m = s_of_c == s
            a3h[:NR, d, s, m] = Af[:, q_of_c[m]]
    a3h = a3h.astype(bf)

    # b3[r, s, o] = Bcat[3o+s, r];  Bcat = lora_B transposed to [768, 96]
    Bcat = Bf.transpose(1, 0, 2).reshape(COUT * K, NR)
    b3h = np.zeros((P, 3, COUT), np.float32)
    b3h[:NR] = Bcat.reshape(COUT, 3, NR).transpose(2, 1, 0)

    # alph[r, smp] per core: scaling*alpha, tasks repeated 24x; zero rows >= 96
    alphh = np.zeros((NCORES, P, S), np.float32)
    rep = np.repeat(af * SCALING, R * K, axis=1)           # [B, 96]
    alphh[:, :NR, :] = rep.reshape(NCORES, S, NR).transpose(0, 2, 1)

    eyeh = np.eye(P, dtype=np.float32).astype(bf)

    return xpad, wth, a3h, b3h, alphh, eyeh


def _in_maps(inputs):
    xpad, wth, a3h, b3h, alphh, eyeh = _host_prep(**inputs)
    return [
        {"xp": np.ascontiguousarray(xpad[c]), "wt": wth, "a3": a3h, "b3": b3h,
         "alph": np.ascontiguousarray(alphh[c]), "eye": eyeh}
        for c in range(NCORES)
    ]


def kernel(x, alphas, W, lora_A, lora_B):
    from concourse.bass_utils import run_bass_kernel_spmd

    if "nc" not in _CACHE:
        _CACHE["nc"] = _build_nc()
    nc = _CACHE["nc"]

    in_maps = _in_maps({"x": x, "alphas": alphas, "W": W,
                        "lora_A": lora_A, "lora_B": lora_B})
    res = run_bass_kernel_spmd(nc, in_maps, list(range(NCORES)))
    out = np.empty((B, COUT, H, W_SP), np.float32)
    for c in range(NCORES):
        out[c * S:(c + 1) * S] = res.results[c]["out"].reshape(S, COUT, H, W_SP)
    return out


# revision 19
# speedup vs baseline: 1.0094x; 1.0094x over previous
# MultiLoraConv2d kernel for 8 trn2 NeuronCores (Bass/Tile, data-parallel over batch).
#
# Math (per sample b):
#   delta_flat[b] = sum_t 2*alphas[b,t] * (lora_B[t] @ lora_A[t])        [768, 768]
#   agg[b] = W + delta_flat[b].reshape(COUT, CIN, 3, 3)                  (flat reinterpret)
#   out[b] = conv2d(x[b], agg[b], pad=1)
#
# Device strategy (per core, S = B/8 samples):
#   - All PE operands bf16: fp32 LDWEIGHTS (224ns) only partially hides under
#     the 213ns N=512 matmul stream (286ns/MM measured); bf16 FWL loads hide
#     fully (216ns/MM cadence measured).
#   - Host pre-lays-out tensors partition-first; LoRA factors regrouped by
#     (d = 3*i + j, s = (c*9+d)//768) so per-sample aggregated conv weights
#     come out of the PE directly in c-major (stationary) layout:
#       dp_d[c, smp, o] = sum_s sum_r a3[d,s,r,c] * (2*alpha_{t(r)} * b3[s,r,o])
#   - +W fold: most dp banks evicted with a DVE tensor-add (dp + wt -> wsf
#     bf16); every ~9th dp instead gets an identity-stationary matmul
#     accumulating W in PSUM and a Scalar-engine copy eviction, balancing
#     PE/DVE/ACT so phase 1 stays PE-bound.
#   - Conv = 18 shifted matmuls (9 taps x 2 cin tiles) accumulated per PSUM
#     bank, x zero-padded (66x66) bf16 in SBUF; evictions alternate
#     Vector/Scalar engines.
import numpy as np

B, T, R, ALPHA = 32, 4, 8, 16
CIN, COUT, K = 256, 256, 3
H = W_SP = 64
SCALING = ALPHA / R
NCORES = 8
S = B // NCORES      # samples per core
NR = T * R * K       # 96 lora rows (padded to 128 partitions)
P = 128
HP = H + 2           # 66 padded
HH = 34              # padded-row half-tile height (rows 0:34 and 32:66)
HSMP = S // 2        # samples per 512-wide delta-matmul half
EYES = {8, 17, 26, 35}  # dp indices evicted via eye-matmul + Scalar copy

_CACHE = {}


def _build_nc():
    import concourse.bacc as bacc
    import concourse.mybir as mybir
    import concourse.tile as tile

    f32 = mybir.dt.float32
    bf16 = mybir.dt.bfloat16

    nc = bacc.Bacc("TRN2", target_bir_lowering=False, debug=False, num_devices=NCORES)

    xp = nc.declare_dram_parameter("xp", [S, 2, P, HP, HP], bf16, isOutput=False)
    wt = nc.declare_dram_parameter("wt", [P, 9, 2, COUT], bf16, isOutput=False)
    a3 = nc.declare_dram_parameter("a3", [P, 9, 3, CIN], bf16, isOutput=False)
    b3 = nc.declare_dram_parameter("b3", [P, 3, COUT], f32, isOutput=False)
    alph = nc.declare_dram_parameter("alph", [P, S], f32, isOutput=False)
    eye = nc.declare_dram_parameter("eye", [P, P], bf16, isOutput=False)
    outd = nc.declare_dram_parameter("out", [S, 2, P, H, W_SP], f32, isOutput=True)

    with tile.TileContext(nc) as tc:
        with tc.tile_pool(name="ws_pool", bufs=2) as ws_pool, \
             tc.tile_pool(name="xt_pool", bufs=8) as xt_pool:
            # per-cin-tile aggregated conv weights: [c, d, half, i, o] bf16
            wsf = [ws_pool.tile([P, 9, 2, 2, COUT], bf16, name="wsf")
                   for _ in range(2)]

            # ---- phase 1: aggregated weights via LoRA matmuls ----
            with tc.tile_pool(name="cst", bufs=1) as cst, \
                 tc.tile_pool(name="wps", bufs=1, space="PSUM") as wps, \
                 tc.tile_pool(name="dps", bufs=7, space="PSUM") as dps:
                a3_sb = cst.tile([P, 9, 3, CIN], bf16)
                b3_sb = cst.tile([P, 3, COUT], f32)
                alph_sb = cst.tile([P, S], f32)
                wt_sb = cst.tile([P, 9, 2, COUT], bf16)
                eye_sb = cst.tile([P, P], bf16)
                warm = cst.tile([P, P], bf16)
                b3s = [cst.tile([P, 3, HSMP, COUT], bf16, name=f"b3s{h}")
                       for h in range(2)]

                # DMA transfers complete strictly in issue order at ~290GB/s
                # (each dma_start fans over all 16 engines). Issue in the
                # order the pipeline consumes: small consts, then per-tap
                # a3/wt pairs (a3 gates the delta matmuls, wt the +W
                # accumulation), then sample-0 x.
                # b3/alph ride the Scalar queue (free at ~5.6us) so the
                # b3s muls can start ~1.5us earlier than via the sync queue.
                nc.scalar.dma_start(b3_sb[:, :, :], b3[:, :, :])
                nc.scalar.dma_start(alph_sb[:, :], alph[:, :])
                nc.sync.dma_start(eye_sb[:, :], eye[:, :])
                nc.sync.dma_start(a3_sb[:, 0:1], a3[:, 0:1])
                nc.sync.dma_start(wt_sb[:, 0:1], wt[:, 0:1])
                nc.sync.dma_start(a3_sb[:, 1:3], a3[:, 1:3])
                nc.sync.dma_start(wt_sb[:, 1:3], wt[:, 1:3])
                nc.sync.dma_start(a3_sb[:, 3:6], a3[:, 3:6])
                nc.sync.dma_start(wt_sb[:, 3:6], wt[:, 3:6])
                nc.sync.dma_start(a3_sb[:, 6:9], a3[:, 6:9])
                nc.sync.dma_start(wt_sb[:, 6:9], wt[:, 6:9])
                # sample-0 x as (ct, h-half) quarters, lower halves first so
                # the first conv bank-groups can start before the rest lands
                xt0 = [[None, None], [None, None]]
                for h in range(2):
                    for ct in range(2):
                        t = xt_pool.tile([P, HH, HP], bf16, name="xt")
                        nc.sync.dma_start(t[:, :, :], xp[0, ct, :, 32 * h:32 * h + HH, :])
                        xt0[ct][h] = t

                # HAM warmup: PE busy from ~6.5us so the clock gate flips
                # to 8/8 and the first-DMA spin-up (~10.5us to b3s ready)
                # is covered. memset is the first GpSimd instruction.
                nc.gpsimd.memset(warm[:, :], 0.0)
                wp = wps.tile([P, P], f32, name="wp")
                for _ in range(40):
                    nc.tensor.matmul(wp[:, :], warm[:, :], warm[:, :],
                                     start=True, stop=True)

                # b3s[half][r, s, smp, o] = b3[r, s, o] * 2*alpha[smp, t(r)]
                # (2x folded into alph host-side). DVE/GpSimd tensor_scalar
                # with an AP scalar hits a ~3.9us/op slow path after the
                # first op, so: half0 as DVE tensor_tensor with a broadcast
                # alpha (s-major, so the first delta matmuls start asap),
                # half1 as Scalar ACTIVATE Copy-with-scale — in parallel.
                # interleave engines by sample so each s-pair completes as
                # the delta-matmul stream reaches it
                for half in range(2):
                    for s in range(3):
                        nc.vector.tensor_mul(
                            b3s[half][:, s, 0, :], b3_sb[:, s, :],
                            alph_sb[:, 2 * half:2 * half + 1]
                            .to_broadcast([P, COUT]))
                        nc.scalar.mul(
                            b3s[half][:, s, 1, :], b3_sb[:, s, :],
                            alph_sb[:, 2 * half + 1:2 * half + 2])

                ev = 0
                for half in range(2):
                    for d in range(9):
                        for ct in range(2):
                            use_eye = ev in EYES
                            dp = dps.tile([P, HSMP, COUT], f32, name="dp")
                            for s in range(3):
                                nc.tensor.matmul(
                                    dp[:, :, :],
                                    a3_sb[:, d, s, ct * P:(ct + 1) * P],
                                    b3s[half][:, s, :, :],
                                    start=(s == 0),
                                    stop=(s == 2 and not use_eye))
                            wtb = wt_sb[:, d, ct, None, :].to_broadcast(
                                [P, HSMP, COUT])
                            dst = wsf[ct][:, d, half, :, :]
                            if use_eye:
                                # += W via identity stationary, wt broadcast
                                # over the sample axis as the moving operand
                                nc.tensor.matmul(
                                    dp[:, :, :], eye_sb[:, :], wtb,
                                    start=False, stop=True)
                                nc.scalar.copy(dst, dp[:, :, :])
                            else:
                                nc.vector.tensor_add(dst, dp[:, :, :], wtb)
                            ev += 1

            # ---- phase 2: per-sample conv, 18 shifted matmuls per psum bank ----
            with tc.tile_pool(name="ob_pool", bufs=4) as ob_pool, \
                 tc.tile_pool(name="cps", bufs=8, space="PSUM") as cps:
                for smp in range(S):
                    half, i = divmod(smp, HSMP)
                    if smp == 0:
                        xts = xt0
                    else:
                        xts = [[None, None], [None, None]]
                        for h in range(2):
                            for ct in range(2):
                                t = xt_pool.tile([P, HH, HP], bf16, name="xt")
                                nc.sync.dma_start(
                                    t[:, :, :], xp[smp, ct, :, 32 * h:32 * h + HH, :])
                                xts[ct][h] = t
                    for ot in range(2):
                        for hb in range(8):
                            pb = cps.tile([P, 8, W_SP], f32, name="pb")
                            first = True
                            for ct in range(2):
                                for d in range(9):
                                    di, dj = divmod(d, 3)
                                    loc = (hb % 4) * 8 + di
                                    nc.tensor.matmul(
                                        pb[:, :, :],
                                        wsf[ct][:, d, half, i,
                                                ot * P:(ot + 1) * P],
                                        xts[ct][hb // 4][:, loc:loc + 8,
                                                         dj:dj + W_SP],
                                        start=first, stop=(ct == 1 and d == 8))
                                    first = False
                            ob = ob_pool.tile([P, 8, W_SP], f32, name="ob")
                            if hb % 2 == 0:
                                nc.vector.tensor_copy(ob[:, :, :], pb[:, :, :])
                            else:
                                nc.scalar.copy(ob[:, :, :], pb[:, :, :])
                            nc.sync.dma_start(
                                outd[smp, ot, :, hb * 8:(hb + 1) * 8, :],
                                ob[:, :, :])
    nc.finalize()
    return nc


def _host_prep(x, alphas, W, lora_A, lora_B):
    """Host-side layout-only transforms (pad/transpose/gather/replicate)."""
    import ml_dtypes
    bf = ml_dtypes.bfloat16

    xf = np.ascontiguousarray(np.asarray(x, dtype=np.float32))
    af = np.asarray(alphas, dtype=np.float32)
    Wf = np.asarray(W, dtype=np.float32)
    Af = np.asarray(lora_A, dtype=np.float32).reshape(NR, CIN * K)   # Acat
    Bf = np.asarray(lora_B, dtype=np.float32)

    # padded x, per core: (S, 2, 128, 66, 66) bf16
    xpad = np.zeros((B, CIN, HP, HP), bf)
    xpad[:, :, 1:-1, 1:-1] = xf.astype(bf)
    xpad = xpad.reshape(NCORES, S, 2, P, HP, HP)

    # base weights c-major, d-major free layout: wt[p, d, ct, o]
    wth = np.ascontiguousarray(
        Wf.reshape(COUT, CIN, 9).transpose(1, 2, 0)        # [c, d, o]
        .reshape(2, P, 9, COUT)                            # [ct, p, d, o]
        .transpose(1, 2, 0, 3)).astype(bf)                 # [p, d, ct, o]

    # a3[r, d, s, c] = Acat[r, c*9+d-768*s] masked; rows padded 96 -> 128
    a3h = np.zeros((P, 9, 3, CIN), np.float32)
    cc = np.arange(CIN)
    for d in range(9):
        q = cc * 9 + d
        s_of_c = q // (CIN * K)
        q_of_c = q % (CIN * K)
        for s in range(3):
            m = s_of_c == s
            a3h[:NR, d, s, m] = Af[:, q_of_c[m]]
    a3h = a3h.astype(bf)

    # b3[r, s, o] = Bcat[3o+s, r];  Bcat = lora_B transposed to [768, 96]
    Bcat = Bf.transpose(1, 0, 2).reshape(COUT * K, NR)
    b3h = np.zeros((P, 3, COUT), np.float32)
    b3h[:NR] = Bcat.reshape(COUT, 3, NR).transpose(2, 1, 0)

    # alph[r, smp] per core: scaling*alpha, tasks repeated 24x; zero rows >= 96
    alphh = np.zeros((NCORES, P, S), np.float32)
    rep = np.repeat(af * SCALING, R * K, axis=1)           # [B, 96]
    alphh[:, :NR, :] = rep.reshape(NCORES, S, NR).transpose(0, 2, 1)

    eyeh = np.eye(P, dtype=np.float32).astype(bf)

    return xpad, wth, a3h, b3h, alphh, eyeh


def _in_maps(inputs):
    xpad, wth, a3h, b3h, alphh, eyeh = _host_prep(**inputs)
    return [
        {"xp": np.ascontiguousarray(xpad[c]), "wt": wth, "a3": a3h, "b3": b3h,
         "alph": np.ascontiguousarray(alphh[c]), "eye": eyeh}
        for c in range(NCORES)
    ]


def kernel(x, alphas, W, lora_A, lora_B):
    from concourse.bass_utils import run_bass_kernel_spmd

    if "nc" not in _CACHE:
        _CACHE["nc"] = _build_nc()
    nc = _CACHE["nc"]

    in_maps = _in_maps({"x": x, "alphas": alphas, "W": W,
                        "lora_A": lora_A, "lora_B": lora_B})
    res = run_bass_kernel_spmd(nc, in_maps, list(range(NCORES)))
    out = np.empty((B, COUT, H, W_SP), np.float32)
    for c in range(NCORES):
        out[c * S:(c + 1) * S] = res.results[c]["out"].reshape(S, COUT, H, W_SP)
    return out
